# revision 5
# baseline (speedup 1.0000x reference)
"""DIEN v2: attention + transposed-state AUGRU scan.

Shapes: B=512, T=256, D=256, H=256; 8 cores, 64 rows/core.

v2 keeps the recurrent state transposed (hT [128, 2(kk), 64b]) so the
per-step PE transposes of v1 disappear and all elementwise work runs on
128 partitions instead of 64. Gate matmuls run in bf16 (12 x N=64), the
u/r input projections are injected into PSUM via an identity matmul,
and the z projection is added on DVE in f32. score is pre-broadcast
along partitions (scoreB) so the h update is pure elementwise.

Per-step chains are split by k-chunk (c) so two independent
ACT->DVE->Pool chains pipeline against each other; ix production for
block t//Tb+2 and keys transposes for t//Tb+2 are interleaved into the
step loop after the chain-critical ops.
"""

from contextlib import ExitStack

import numpy as np

import concourse.bass as bass
import concourse.mybir as mybir
import concourse.tile as tile
from concourse import bacc
from concourse.bass_utils import run_bass_kernel_spmd
from concourse.masks import make_identity

F32 = mybir.dt.float32
F32R = mybir.dt.float32r
BF16 = mybir.dt.bfloat16
I32 = mybir.dt.int32
AF = mybir.ActivationFunctionType
OP = mybir.AluOpType

NEG_INF = -2.0**32 + 1.0

B, T_FULL, D, H = 512, 256, 256, 256
N_CORES = 8
B_LOC = B // N_CORES  # 64
G3 = 3 * H  # 768


def build_program(T=T_FULL, Tb=16, stage="full"):
    nc = bacc.Bacc(None, target_bir_lowering=False)

    q_d = nc.dram_tensor("query", [B_LOC, D], F32, kind="ExternalInput")
    k_d = nc.dram_tensor("keys", [B_LOC, T, D], F32, kind="ExternalInput")
    sl_d = nc.dram_tensor("seq_len", [B_LOC, 1], I32, kind="ExternalInput")
    wa_d = nc.dram_tensor("w_att", [D, D], F32, kind="ExternalInput")
    w_d = nc.dram_tensor("w", [G3, D], F32, kind="ExternalInput")
    u_d = nc.dram_tensor("u", [G3, H], F32, kind="ExternalInput")
    bu_d = nc.dram_tensor("bu", [1, H], F32, kind="ExternalInput")
    br_d = nc.dram_tensor("br", [1, H], F32, kind="ExternalInput")
    bh_d = nc.dram_tensor("bh", [1, H], F32, kind="ExternalInput")
    out_d = nc.dram_tensor("out", [B_LOC, H], F32, kind="ExternalOutput")
    st_dram = nc.dram_tensor("st_scratch", [1, 2, 128, B_LOC], BF16, kind="Internal")

    n_blocks = T // Tb
    R = lambda ap: ap.bitcast(F32R)

    with tile.TileContext(nc) as tc, ExitStack() as ctx:
        const = ctx.enter_context(tc.tile_pool(name="const", bufs=1))
        kbpool = ctx.enter_context(tc.tile_pool(name="kb", bufs=2))
        kstpool = ctx.enter_context(tc.tile_pool(name="kst", bufs=2))
        ixrupool = ctx.enter_context(tc.tile_pool(name="ixru", bufs=2))
        izpool = ctx.enter_context(tc.tile_pool(name="iz", bufs=2))
        sbpool = ctx.enter_context(tc.tile_pool(name="sb", bufs=2))
        step_p = ctx.enter_context(tc.tile_pool(name="step", bufs=3))
        pers = ctx.enter_context(tc.tile_pool(name="persist", bufs=1))
        ps_g = ctx.enter_context(tc.tile_pool(name="ps_g", bufs=2, space="PSUM"))
        ps_ix = ctx.enter_context(tc.tile_pool(name="ps_ix", bufs=2, space="PSUM"))
        ps_tr = ctx.enter_context(tc.tile_pool(name="ps_tr", bufs=2, space="PSUM"))

        # ---------------- constants / weight prep ----------------
        uT_st = const.tile([128, 2, G3], F32)  # [d_k, kk, g] internal order
        wT_st = const.tile([128, 2, G3], F32R)
        wa = const.tile([128, 2, D], F32R)
        qT = const.tile([128, 2, B_LOC], F32R)
        qp_sb = const.tile([B_LOC, D], F32)
        uTb = const.tile([128, 2, G3], BF16)
        i64f = const.tile([B_LOC, B_LOC], F32)
        i128f = const.tile([128, 128], F32)
        Ib = const.tile([128, 128], BF16)
        ones_f = const.tile([128, Tb * B_LOC], F32)  # [128, 1024]
        ones_r = const.tile([128, Tb * B_LOC], F32R)
        ones1b = const.tile([1, 128], BF16)
        bias_cols = const.tile([128, 6], F32)
        diag_f = const.tile([128, 6, 128], F32)
        diagb = const.tile([128, 6, 128], F32R)
        iota_g = const.tile([B_LOC, T], F32)
        iota_t = const.tile([B_LOC, T], F32)
        neg_inf = const.tile([B_LOC, T], F32)
        seq_sb = const.tile([B_LOC, 1], I32)
        seq_f = const.tile([B_LOC, 1], F32)
        logits = const.tile([B_LOC, T], F32)
        logits_m = const.tile([B_LOC, T], F32)
        exps = const.tile([B_LOC, T], F32)
        score = const.tile([B_LOC, T], F32)
        scoreTb = const.tile([128, 2, B_LOC], BF16)
        sflat = const.tile([1, T * B_LOC], BF16)
        nmax = const.tile([B_LOC, 1], F32)
        sumexp = const.tile([B_LOC, 1], F32)
        recd = const.tile([B_LOC, 1], F32)

        make_identity(nc, i64f[:, :])
        make_identity(nc, i128f[:, :])
        nc.scalar.copy(out=Ib[:, :], in_=i128f[:, :])
        nc.vector.memset(ones_f[:, :], 1.0)
        nc.scalar.copy(out=ones_r[:, :], in_=ones_f[:, :])
        nc.scalar.copy(out=ones1b[:, :], in_=ones_f[0:1, 0:128])
        nc.vector.memset(neg_inf[:, :], NEG_INF)
        nc.gpsimd.iota(
            iota_g[:, :], pattern=[[1, T]], base=0, channel_multiplier=0,
            allow_small_or_imprecise_dtypes=True,
        )
        nc.vector.tensor_copy(out=iota_t[:, :], in_=iota_g[:, :])
        nc.sync.dma_start(out=seq_sb[:, :], in_=sl_d[:, :])
        nc.vector.tensor_copy(out=seq_f[:, :], in_=seq_sb[:, :])

        # weights, internal gate slot order [r|u|z] <- ref rows [u|r|z]
        slot_rows = [(0, H, H), (1, 0, H), (2, 2 * H, H)]  # (slot, row0, n)
        uT_dram = u_d[:, :].rearrange("g d -> d g")
        wT_dram = w_d[:, :].rearrange("g d -> d g")
        for kk in range(2):
            for slot, r0, n in slot_rows:
                nc.sync.dma_start(
                    out=uT_st[:, kk, slot * H : slot * H + n],
                    in_=uT_dram[kk * 128 : (kk + 1) * 128, r0 : r0 + n],
                )
                nc.sync.dma_start(
                    out=wT_st[:, kk, slot * H : slot * H + n],
                    in_=wT_dram[kk * 128 : (kk + 1) * 128, r0 : r0 + n].bitcast(F32R),
                )
            nc.sync.dma_start(
                out=wa[:, kk, :],
                in_=wa_d[:, :]
                .rearrange("i j -> j i")[kk * 128 : (kk + 1) * 128, :]
                .bitcast(F32R),
            )
            nc.sync.dma_start(
                out=qT[:, kk, :],
                in_=q_d[:, :]
                .rearrange("b d -> d b")[kk * 128 : (kk + 1) * 128, :]
                .bitcast(F32R),
            )
        nc.scalar.copy(
            out=uTb[:, :, :].rearrange("p k g -> p (k g)"),
            in_=uT_st[:, :, :].rearrange("p k g -> p (k g)"),
        )

        # per-(slot, c) bias columns on partitions, then diag matrices
        for slot, b_dram in [(0, br_d), (1, bu_d), (2, bh_d)]:
            for c in range(2):
                nc.sync.dma_start(
                    out=bias_cols[:, 2 * slot + c : 2 * slot + c + 1],
                    in_=b_dram[:, c * 128 : (c + 1) * 128].rearrange("o h -> h o"),
                )
        for j in range(6):
            nc.vector.tensor_scalar_mul(
                diag_f[:, j, :], i128f[:, :], bias_cols[:, j : j + 1]
            )
        nc.scalar.copy(
            out=diagb[:, :, :].rearrange("p j k -> p (j k)"),
            in_=diag_f[:, :, :].rearrange("p j k -> p (j k)"),
        )

        # ---------------- q_proj ----------------
        ps_qp = ps_ix.tile([B_LOC, D], F32, tag="ixps", name="ps_qp")
        for kk in range(2):
            nc.tensor.matmul(
                ps_qp[:, :], R(qT[:, kk, :]), R(wa[:, kk, :]),
                start=(kk == 0), stop=(kk == 1),
            )
        nc.scalar.copy(out=qp_sb[:, :], in_=ps_qp[:, :])

        # ---------------- block production helpers ----------------
        kb_tiles = {}
        kst_tiles = {}
        ixru_tiles = {}
        iz_tiles = {}
        sb_tiles = {}

        def load_kblock(blk):
            kb = kbpool.tile([B_LOC, Tb, D], F32, tag="kb", name=f"kb{blk}")
            nc.sync.dma_start(out=kb[:, :, :], in_=k_d[:, blk * Tb : (blk + 1) * Tb, :])
            kb_tiles[blk] = kb
            return kb

        def emit_transpose_group(blk, grp):
            """Transpose 4 timesteps of kb[blk] and append them to kst[blk].

            grp in 0..3; each group covers taus 4*grp..4*grp+3.
            """
            if grp == 0:
                kst_tiles[blk] = kstpool.tile(
                    [128, 2, Tb, B_LOC], F32R, tag="kst", name=f"kst{blk}"
                )
            kb = kb_tiles[blk]
            kst = kst_tiles[blk]
            ktr = ps_tr.tile([128, 4, 2, B_LOC], F32, tag="tr", name=f"ktr{blk}_{grp}")
            t0 = 4 * grp
            for i in range(4):
                for kk in range(2):
                    nc.tensor.transpose(
                        ktr[:, i, kk, :],
                        kb[:, t0 + i, kk * 128 : (kk + 1) * 128],
                        i64f[:, :],
                    )
            nc.scalar.copy(
                out=kst[:, :, t0 : t0 + 4, :],
                in_=ktr[:, :, :, :].rearrange("p t k b -> p k t b"),
            )

        def emit_ix_prod(blk, sc):
            """One (slot, c) production: full-block ix PSUM + copy out.

            sc in 0..5 -> slot = sc // 2, c = sc % 2.
            """
            slot, c = sc // 2, sc % 2
            if sc == 0:
                ixru_tiles[blk] = ixrupool.tile(
                    [128, 2, Tb, 2, B_LOC], BF16, tag="ixru", name=f"ixru{blk}"
                )
                iz_tiles[blk] = izpool.tile(
                    [128, 2, Tb, B_LOC], F32, tag="iz", name=f"iz{blk}"
                )
            kst = kst_tiles[blk]
            ixps = ps_ix.tile([128, Tb * B_LOC], F32, tag="ixps", name=f"ixps{blk}_{sc}")
            col0 = slot * 256 + c * 128
            half = Tb * B_LOC // 2  # psum bank limit: one matmul <= 512 f32
            for hf in range(2):
                sl = slice(hf * half, (hf + 1) * half)
                for kk in range(2):
                    nc.tensor.matmul(
                        ixps[:, sl],
                        wT_st[:, kk, col0 : col0 + 128],
                        kst[:, kk, hf * 8 : (hf + 1) * 8, :].rearrange(
                            "p t b -> p (t b)"
                        ),
                        start=(kk == 0), stop=False,
                    )
                # bias: out[g', n] += bias[g']  (diag lhsT x ones rhs)
                nc.tensor.matmul(
                    ixps[:, sl], diagb[:, sc, :], ones_r[:, 0:half],
                    start=False, stop=True,
                )
            if slot < 2:
                nc.scalar.copy(
                    out=ixru_tiles[blk][:, c, :, slot, :],
                    in_=ixps[:, :].rearrange("p (t b) -> p t b", t=Tb),
                )
            else:
                nc.scalar.copy(
                    out=iz_tiles[blk][:, c, :, :],
                    in_=ixps[:, :].rearrange("p (t b) -> p t b", t=Tb),
                )

        def emit_scoreB_quarter(blk, qr):
            """Broadcast score rows for 4 timesteps (x2 c copies)."""
            if qr == 0:
                sb_tiles[blk] = sbpool.tile(
                    [128, Tb, 2, B_LOC], F32, tag="sb", name=f"sb{blk}"
                )
            sbps = ps_tr.tile([128, 4, 2, B_LOC], F32, tag="tr", name=f"sbps{blk}_{qr}")
            for i in range(4):
                t = blk * Tb + qr * 4 + i
                row = sflat[0:1, t * B_LOC : (t + 1) * B_LOC]
                for c in range(2):
                    nc.tensor.matmul(
                        sbps[:, i, c, :], ones1b[:, :], row,
                        start=(i == 0 and c == 0), stop=(i == 3 and c == 1),
                    )
            nc.scalar.copy(
                out=sb_tiles[blk][:, qr * 4 : qr * 4 + 4, :, :].rearrange(
                    "p t c b -> p (t c b)"
                ),
                in_=sbps[:, :, :, :].rearrange("p t c b -> p (t c b)"),
            )

        # ---------------- attention ----------------
        for blk in range(n_blocks):
            kb = load_kblock(blk)
            for tau in range(Tb):
                t = blk * Tb + tau
                scr = step_p.tile([B_LOC, D], F32, tag="scr", name=f"scr{t}")
                nc.vector.scalar_tensor_tensor(
                    out=scr[:, :],
                    in0=kb[:, tau, :],
                    scalar=1.0,
                    in1=qp_sb[:, :],
                    op0=OP.bypass,
                    op1=OP.mult,
                    accum_out=logits[:, t : t + 1],
                )
            if blk < 2:
                for grp in range(4):
                    emit_transpose_group(blk, grp)

        mask = step_p.tile([B_LOC, T], I32, tag="mask")
        nc.vector.tensor_scalar(
            out=mask[:, :], in0=iota_t[:, :], scalar1=seq_f[:, :], scalar2=None,
            op0=OP.is_lt,
        )
        nc.vector.select(
            out=logits_m[:, :], mask=mask[:, :], on_true=logits[:, :],
            on_false=neg_inf[:, :],
        )
        nc.vector.tensor_reduce(
            out=nmax[:, :], in_=logits_m[:, :], axis=mybir.AxisListType.X,
            op=OP.max, negate=True,
        )
        nc.scalar.activation(
            out=exps[:, :], in_=logits_m[:, :], func=AF.Exp,
            bias=nmax[:, :], scale=1.0, accum_out=sumexp[:, :],
        )
        nc.vector.reciprocal(out=recd[:, :], in_=sumexp[:, :])
        nc.vector.tensor_scalar_mul(score[:, :], exps[:, :], recd[:, :])

        if stage == "attn":
            nc.sync.dma_start(out=out_d[:, 0:T], in_=score[:, :])
            nc.finalize()
            return nc

        # score^T in bf16 for the broadcast matmuls
        strps = ps_tr.tile([128, 2, B_LOC], F32, tag="tr", name="strps")
        for c in range(2):
            nc.tensor.transpose(
                strps[:, c, :], score[:, c * 128 : (c + 1) * 128], i64f[:, :]
            )
        nc.scalar.copy(
            out=scoreTb[:, :, :].rearrange("p c b -> p (c b)"),
            in_=strps[:, :, :].rearrange("p c b -> p (c b)"),
        )
        # bounce score^T through DRAM so every row t lands in partition 0's
        # free dim (matmul rhs base-partition restriction)
        nc.sync.dma_start(
            out=st_dram[0, :, :, :].rearrange("c p b -> p c b"),
            in_=scoreTb[:, :, :],
        )
        nc.sync.dma_start(
            out=sflat[0:1, :].rearrange("o (c p b) -> o c p b", c=2, p=128),
            in_=st_dram[:, :, :, :],
        )

        # prologue: ix + scoreB for blocks 0,1; kb for block 2
        load_kblock(2)
        for blk in range(2):
            for sc in range(6):
                emit_ix_prod(blk, sc)
            for qr in range(4):
                emit_scoreB_quarter(blk, qr)

        # ---------------- scan state ----------------
        hT = pers.tile([128, 2, B_LOC], F32)
        hTb = pers.tile([128, 2, B_LOC], BF16)
        nc.vector.memset(hT[:, :, :].rearrange("p c b -> p (c b)"), 0.0)
        nc.vector.tensor_copy(
            out=hTb[:, :, :].rearrange("p c b -> p (c b)"),
            in_=hT[:, :, :].rearrange("p c b -> p (c b)"),
        )

        # ---------------- scan ----------------
        for t in range(T):
            blk, jj = t // Tb, t % Tb
            ixru = ixru_tiles[blk]
            iz = iz_tiles[blk]
            sb = sb_tiles[blk]

            # --- chain-critical: gate matmuls ---
            # G is one psum bank; exactly one start (claims + lazy-zeroes the
            # whole 2KB zero region) and one stop across all 14 matmuls.
            G = ps_g.tile([128, 2, 3, B_LOC], F32, tag="g", name=f"g{t}")
            for c in range(2):
                nc.tensor.matmul(
                    G[:, c, 0:2, :].rearrange("p s b -> p (s b)"),
                    Ib[:, :],
                    ixru[:, c, jj, :, :].rearrange("p s b -> p (s b)"),
                    start=(c == 0), stop=False,
                )
            for kk in range(2):
                for c in range(2):
                    for slot in range(3):
                        col0 = slot * 256 + c * 128
                        nc.tensor.matmul(
                            G[:, c, slot, :],
                            uTb[:, kk, col0 : col0 + 128],
                            hTb[:, kk, :],
                            start=False,
                            stop=(kk == 1 and c == 1 and slot == 2),
                        )

            # --- per-c chains ---
            # Tile's sem waits round up to the latest-emitted producer per
            # engine, so emission order IS the dependency encoding: each
            # consumer is emitted immediately after its true producer.
            t1s, dd = {}, {}
            sig_rf = step_p.tile([128, 2, B_LOC], F32, tag="sigr", name=f"sr{t}")
            nc.scalar.activation(
                out=sig_rf[:, :, :], in_=G[:, :, 0, :], func=AF.Sigmoid
            )
            zzf = step_p.tile([128, 2, B_LOC], F32, tag="zz", name=f"zz{t}")
            nc.vector.tensor_tensor(
                out=zzf[:, 0, :], in0=G[:, 0, 2, :], in1=sig_rf[:, 0, :], op=OP.mult
            )
            nc.vector.tensor_tensor(
                out=zzf[:, 0, :], in0=zzf[:, 0, :], in1=iz[:, 0, jj, :], op=OP.add
            )
            nc.vector.tensor_tensor(
                out=zzf[:, 1, :], in0=G[:, 1, 2, :], in1=sig_rf[:, 1, :], op=OP.mult
            )
            nc.vector.tensor_tensor(
                out=zzf[:, 1, :], in0=zzf[:, 1, :], in1=iz[:, 1, jj, :], op=OP.add
            )
            ztf = step_p.tile([128, 2, B_LOC], F32, tag="zt", name=f"zt{t}")
            nc.scalar.activation(out=ztf[:, :, :], in_=zzf[:, :, :], func=AF.Tanh)
            dd[0] = step_p.tile([128, B_LOC], F32, tag="dd0", name=f"dd{t}_0")
            nc.gpsimd.tensor_sub(dd[0][:, :], ztf[:, 0, :], hT[:, 0, :])
            dd[1] = step_p.tile([128, B_LOC], F32, tag="dd1", name=f"dd{t}_1")
            nc.gpsimd.tensor_sub(dd[1][:, :], ztf[:, 1, :], hT[:, 1, :])
            sig_u = step_p.tile([128, 2, B_LOC], F32, tag="sigu", name=f"su{t}")
            nc.scalar.activation(out=sig_u[:, :, :], in_=G[:, :, 1, :], func=AF.Sigmoid)
            t1s[0] = step_p.tile([128, B_LOC], F32, tag="t10", name=f"t1{t}_0")
            nc.gpsimd.tensor_mul(t1s[0][:, :], sig_u[:, 0, :], sb[:, jj, 0, :])
            ee0 = step_p.tile([128, B_LOC], F32, tag="ee0", name=f"ee{t}_0")
            nc.vector.tensor_tensor(
                out=ee0[:, :], in0=t1s[0][:, :], in1=dd[0][:, :], op=OP.mult
            )
            nc.vector.tensor_tensor(
                out=hT[:, 0, :], in0=hT[:, 0, :], in1=ee0[:, :], op=OP.add
            )
            nc.vector.tensor_copy(out=hTb[:, 0, :], in_=hT[:, 0, :])
            t1s[1] = step_p.tile([128, B_LOC], F32, tag="t11", name=f"t1{t}_1")
            nc.gpsimd.tensor_mul(t1s[1][:, :], sig_u[:, 1, :], sb[:, jj, 1, :])
            ee1 = step_p.tile([128, B_LOC], F32, tag="ee1", name=f"ee{t}_1")
            nc.vector.tensor_tensor(
                out=ee1[:, :], in0=t1s[1][:, :], in1=dd[1][:, :], op=OP.mult
            )
            nc.vector.tensor_tensor(
                out=hT[:, 1, :], in0=hT[:, 1, :], in1=ee1[:, :], op=OP.add
            )
            nc.vector.tensor_copy(out=hTb[:, 1, :], in_=hT[:, 1, :])

            # --- interleaved production for future blocks ---
            if jj == 0 and blk + 3 < n_blocks:
                load_kblock(blk + 3)
            if jj < 8 and jj % 2 == 0 and blk + 2 < n_blocks:
                emit_transpose_group(blk + 2, jj // 2)
            if 8 <= jj < 14 and 2 <= blk + 1 < n_blocks:
                emit_ix_prod(blk + 1, jj - 8)
            if jj >= 14 and 2 <= blk + 1 < n_blocks:
                emit_scoreB_quarter(blk + 1, 2 * (jj - 14))
                emit_scoreB_quarter(blk + 1, 2 * (jj - 14) + 1)

        # ---------------- epilogue: transpose hT -> out ----------------
        otr = ps_tr.tile([B_LOC, 2, 128], F32, tag="tr", name="otr")
        for c in range(2):
            nc.tensor.transpose(otr[:, c, :], hT[:, c, :], i128f[:, :])
        out_sb = pers.tile([B_LOC, D], F32)
        nc.scalar.copy(
            out=out_sb[:, :], in_=otr[:, :, :].rearrange("p c h -> p (c h)")
        )
        nc.sync.dma_start(out=out_d[:, :], in_=out_sb[:, :])

    nc.finalize()
    return nc


def _shard_inputs(query, keys, seq_len, w_att, w, u, bu, br, bh, T=T_FULL):
    in_maps = []
    for c in range(N_CORES):
        s = slice(c * B_LOC, (c + 1) * B_LOC)
        in_maps.append(
            {
                "query": np.ascontiguousarray(query[s], dtype=np.float32),
                "keys": np.ascontiguousarray(keys[s, :T], dtype=np.float32),
                "seq_len": np.ascontiguousarray(
                    seq_len[s].reshape(B_LOC, 1), dtype=np.int32
                ),
                "w_att": np.ascontiguousarray(w_att, dtype=np.float32),
                "w": np.ascontiguousarray(w, dtype=np.float32),
                "u": np.ascontiguousarray(u, dtype=np.float32),
                "bu": np.ascontiguousarray(bu.reshape(1, -1), dtype=np.float32),
                "br": np.ascontiguousarray(br.reshape(1, -1), dtype=np.float32),
                "bh": np.ascontiguousarray(bh.reshape(1, -1), dtype=np.float32),
            }
        )
    return in_maps


_CACHED = {}


def run_on_device(inputs, T=T_FULL, Tb=16, trace=False, **build_kw):
    key = (T, Tb, tuple(sorted(build_kw.items())))
    if key not in _CACHED:
        _CACHED[key] = build_program(T=T, Tb=Tb, **build_kw)
    nc = _CACHED[key]
    in_maps = _shard_inputs(**inputs, T=T)
    res = run_bass_kernel_spmd(
        nc, in_maps, core_ids=list(range(N_CORES)), trace=trace
    )
    out = np.concatenate([r["out"] for r in res.results], axis=0)
    return out, res


def kernel(query, keys, seq_len, w_att, w, u, bu, br, bh):
    out, _ = run_on_device(
        dict(
            query=query, keys=keys, seq_len=seq_len, w_att=w_att, w=w, u=u,
            bu=bu, br=br, bh=bh,
        )
    )
    return out.astype(np.float32)
